# revision 2
# baseline (speedup 1.0000x reference)
"""LookAheadMask kernel for Trainium2.

out[b, r, c] = 1.0 if c > r else x[b, r, c], for x of shape (8, 4096, 4096) f32.

Sharding: batch dim across 8 NeuronCores (data parallel, no communication).

Per-core plan (matrix is S x S, S=4096, row-blocks of P=128), raw bass
(explicit engines + semaphores; the Tile drain would exceed walrus's
sync-wait-slot limit with this many independent DMAs):

  - strictly-lower region (cols < block start): 31 direct DRAM->DRAM copy
    DMAs on the SP (sync) HWDGE ring
  - strictly-upper region (cols >= block end): 31 DMAs from an SBUF ones
    tile, also SP ring (no HBM read for that half)
  - the 32 diagonal 128x128 blocks: one 3D-strided gather DMA on the ACT
    (scalar) ring into SBUF [128, 32*128], one gpsimd affine_select
    (keep x where row >= col-within-block, else 1.0), one scatter back.
    Separate ring so the small-descriptor diag traffic overlaps the bulk.

HBM traffic/core: ~33 MiB read + 64 MiB write vs 128 MiB naive.
"""

import numpy as np

from concourse import bass, mybir
from concourse.bass_utils import run_bass_kernel_spmd

S = 4096
P = 128
NB = S // P  # 32
N_CORES = 8

_cached_nc = None


def _build():
    global _cached_nc
    if _cached_nc is not None:
        return _cached_nc

    nc = bass.Bass()
    x = nc.dram_tensor("x", [S, S], mybir.dt.float32, kind="ExternalInput")
    out = nc.dram_tensor("out", [S, S], mybir.dt.float32, kind="ExternalOutput")

    # Diagonal-block view: [row-in-block(128), block(32), col-in-block(128)],
    # block b starts at element offset b*(P*S + P). Strides in elements.
    diag_pairs = [[S, P], [P * S + P, NB], [1, P]]

    with (
        nc.Block() as block,
        nc.semaphore("dsem") as dsem,  # SP-ring DMA completions
        nc.semaphore("gsem") as gsem,  # diag gather done
        nc.semaphore("ssem") as ssem,  # diag scatter done
        nc.semaphore("msem") as msem,  # ones memset done
        nc.semaphore("asem") as asem,  # affine_select done
        nc.sbuf_tensor("ones", [P, S], mybir.dt.float32) as ones,
        nc.sbuf_tensor("diag_in", [P, S], mybir.dt.float32) as diag_in,
        nc.sbuf_tensor("diag_out", [P, S], mybir.dt.float32) as diag_out,
    ):

        @block.vector
        def _(vector: bass.BassVectorEngine):
            vector.memset(ones[:, :], 1.0).then_inc(msem, 1)

        @block.scalar
        def _(scalar: bass.BassEngine):
            scalar.dma_start(
                out=diag_in[:, :], in_=bass.AP(x, 0, diag_pairs)
            ).then_inc(gsem, 16)
            scalar.wait_ge(asem, 1)
            scalar.dma_start(
                out=bass.AP(out, 0, diag_pairs), in_=diag_out[:, :]
            ).then_inc(ssem, 16)

        @block.gpsimd
        def _(gpsimd: bass.BassGpSimd):
            gpsimd.wait_ge(gsem, 16)
            # iota[p, c] = p - (c % 128); keep x where >= 0 (at/below diag)
            gpsimd.affine_select(
                out=diag_out[:, :],
                in_=diag_in[:, :],
                pattern=[[0, NB], [-1, P]],
                base=0,
                channel_multiplier=1,
                compare_op=mybir.AluOpType.is_ge,
                fill=1.0,
            ).then_inc(asem, 1)

        @block.sync
        def _(sync: bass.BassEngine):
            n = 0
            for i in range(1, NB):  # strictly-lower copies, DRAM->DRAM
                r0 = i * P
                sync.dma_start(
                    out=out[r0 : r0 + P, 0:r0], in_=x[r0 : r0 + P, 0:r0]
                ).then_inc(dsem, 16)
                n += 1
            sync.wait_ge(msem, 1)
            for i in range(NB - 1):  # strictly-upper ones
                r0 = i * P
                w = S - r0 - P
                sync.dma_start(
                    out=out[r0 : r0 + P, r0 + P : S], in_=ones[:, :w]
                ).then_inc(dsem, 16)
                n += 1
            sync.wait_ge(dsem, 16 * n)
            sync.wait_ge(ssem, 16)

    _cached_nc = nc
    return nc


def _run(x_full: np.ndarray, trace: bool = False):
    nc = _build()
    x_full = np.asarray(x_full, dtype=np.float32)
    in_maps = [{"x": x_full[i]} for i in range(N_CORES)]
    res = run_bass_kernel_spmd(nc, in_maps, list(range(N_CORES)), trace=trace)
    out = np.stack([res.results[i]["out"] for i in range(N_CORES)], axis=0)
    return out, res


def kernel(x: np.ndarray) -> np.ndarray:
    out, _ = _run(x, trace=False)
    return out
